# revision 3
# baseline (speedup 1.0000x reference)
"""Trainium2 Bass kernel for nn_LocalNetwork (avgpool3d -> 3x LocallyConnected1D -> upsample3d).

Sharding: pure data parallelism — batch 256 split as 32 per core across 8 cores.

Per-core layout (B_loc=32 batches, 4 groups of 8):
  partition p = (bl, dslice)  [8 x 15 = 120 partitions]
  Every DMA descriptor covers a 32KB-contiguous DRAM run (one (h,w) slice),
  vs 2KB runs in the row-partitioned layout — this is what the DMA engines
  need to hit full throughput, and it cuts descriptor-generation work on
  the sync engine ~16x.

  - avg-pool over (hs,ws): one fused tensor_reduce XY -> P2 [120, 512]
  - depth pool + depth-conv taps: 3 matmuls [120 -> 40] with 0/(1/48)
    matrices (fuses the /48 mean scale and the +/-1 depth shifts)
  - depth/lon/lat convs: free-axis mult-adds on [40, 512] tiles with
    per-(partition, free) weight tiles precomputed on host
  - upsample: h,w expansion via broadcast copies on scalar/gpsimd;
    depth x3 replication is FREE — three store DMAs read the same
    [40, 8192] SBUF tile into interleaved DRAM depth slices.
"""

import numpy as np

import concourse.bass as bass
import concourse.mybir as mybir
from concourse import bacc
from concourse.bass_utils import run_bass_kernel_spmd
from concourse.tile import TileContext

F32 = mybir.dt.float32
ADD = mybir.AluOpType.add
MULT = mybir.AluOpType.mult
RELU = mybir.ActivationFunctionType.Relu

N_CORES = 8
B = 256
B_CORE = 32          # batches per core
G = 4                # groups per core
B_GRP = 8            # batches per group
CORE_ELEMS = B_CORE * 15 * 64 * 128  # 3,932,160
BSTRIDE = 15 * 64 * 128              # 122,880
SLICE = 64 * 128                     # 8192 elems = one (h,w) plane = 32KB


def _pack_consts(w_depth, b_depth, w_lon, b_lon, w_lat, b_lat):
    """Returns (mm [120,128] f32, wts [40,6144] f32).

    mm: three matmul lhsT tiles [120,40] (cols 0:40 dn / 40:80 mid / 80:120 up)
        out[q=(bl,dp), f] = sum_p lhsT[p=(bl,dsl), q] * P2[p, f]
        coefficient 1/48 folds the avg-pool mean.
    wts: 12 x [40,512] conv weight/bias tiles, p=(bl,dp), f=(ho,wo).
    """
    mm = np.zeros((120, 128), np.float32)
    for bl in range(8):
        for dsl in range(15):
            p = bl * 15 + dsl
            grp = dsl // 3
            for col0, dp in ((0, grp + 1), (40, grp), (80, grp - 1)):
                if 0 <= dp <= 4:
                    mm[p, col0 + bl * 5 + dp] = 1.0 / 48.0

    dp = np.arange(5)[:, None, None]
    ho = np.arange(16)[None, :, None]
    wo = np.arange(32)[None, None, :]
    ld = wo * 112 + ho * 7 + (dp + 1)     # depth seq index (5,16,32)
    ll = dp * 544 + ho * 34 + (wo + 1)    # lon
    lt = dp * 576 + wo * 18 + (ho + 1)    # lat

    def tile(vec, idx):
        t = np.broadcast_to(np.asarray(vec)[idx][None], (8, 5, 16, 32))
        return t.reshape(40, 512)

    cols = []
    for j in range(3):
        cols.append(tile(np.asarray(w_depth)[:, j], ld))
    cols.append(tile(b_depth, ld))
    for j in range(3):
        cols.append(tile(np.asarray(w_lon)[:, j], ll))
    cols.append(tile(b_lon, ll))
    for j in range(3):
        cols.append(tile(np.asarray(w_lat)[:, j], lt))
    cols.append(tile(b_lat, lt))
    wts = np.ascontiguousarray(np.concatenate(cols, axis=1), dtype=np.float32)
    return mm, wts


def build_nc(reps: int = 1) -> bass.Bass:
    nc = bacc.Bacc("TRN2", target_bir_lowering=False, debug=False)
    x = nc.dram_tensor("x", [CORE_ELEMS], F32, kind="ExternalInput")
    mmc = nc.dram_tensor("mm", [120, 128], F32, kind="ExternalInput")
    wtc = nc.dram_tensor("wts", [40, 6144], F32, kind="ExternalInput")
    y = nc.dram_tensor("y", [CORE_ELEMS], F32, kind="ExternalOutput")

    with TileContext(nc) as tc:
        with (
            tc.tile_pool(name="cpool", bufs=1) as cpool,
            tc.tile_pool(name="inp", bufs=2) as inp,
            tc.tile_pool(name="outp", bufs=2) as outp,
            tc.tile_pool(name="work", bufs=2) as work,
            tc.tile_pool(name="psum", bufs=2, space="PSUM") as psum,
        ):
            MM = cpool.tile([120, 128], F32)
            WT = cpool.tile([40, 6144], F32)
            nc.sync.dma_start(MM[:], mmc[:])
            nc.sync.dma_start(WT[:], wtc[:])
            w = lambda i: WT[:, i * 512:(i + 1) * 512]
            wd0, wd1, wd2, bd = (w(i) for i in range(4))
            vl0, vl1, vl2, blon = (w(i) for i in range(4, 8))
            ul0, ul1, ul2, blat = (w(i) for i in range(8, 12))

            for g in range(G * reps):
                g = g % G
                off = g * B_GRP * BSTRIDE

                # ---- load: X[p=(bl,dsl), (h,w)] — 32KB contiguous runs ----
                X = inp.tile([120, SLICE], F32)
                nc.sync.dma_start(
                    X[:], bass.AP(x, off, [[BSTRIDE, 8], [SLICE, 15], [1, SLICE]]))

                # ---- h,w avg-pool (sum): fused reduce over (hs, ws) ----
                P2 = work.tile([120, 512], F32)
                Xv = X[:].rearrange("p (ho hs wo ws) -> p ho wo hs ws",
                                    ho=16, hs=4, wo=32, ws=4)
                nc.vector.tensor_reduce(
                    P2[:].rearrange("p (ho wo) -> p ho wo", ho=16),
                    Xv, mybir.AxisListType.XY, ADD)

                # ---- depth pool (/48) + conv taps via matmul [120->40] ----
                Sdn = psum.tile([40, 512], F32)
                S0 = psum.tile([40, 512], F32)
                Sup = psum.tile([40, 512], F32)
                nc.tensor.matmul(Sdn[:], MM[:, 0:40], P2[:], start=True, stop=True)
                nc.tensor.matmul(S0[:], MM[:, 40:80], P2[:], start=True, stop=True)
                nc.tensor.matmul(Sup[:], MM[:, 80:120], P2[:], start=True, stop=True)

                # ---- depth conv ----
                m = work.tile([40, 512], F32)
                m2 = work.tile([40, 512], F32)
                nc.vector.tensor_tensor(m[:], wd0, Sdn[:], MULT)
                nc.vector.tensor_tensor(m2[:], wd1, S0[:], MULT)
                nc.vector.tensor_tensor(m[:], m[:], m2[:], ADD)
                nc.vector.tensor_tensor(m2[:], wd2, Sup[:], MULT)
                nc.vector.tensor_tensor(m[:], m[:], m2[:], ADD)
                nc.vector.tensor_tensor(m[:], m[:], bd, ADD)
                # relu into lon-padded tile Ydp[p, ho*34 + (wo+1)]
                Ydp = work.tile([40, 544], F32)
                Ydpv = Ydp[:].rearrange("p (ho wp) -> p ho wp", ho=16, wp=34)
                nc.gpsimd.memset(Ydpv[:, :, 0], 0)
                nc.gpsimd.memset(Ydpv[:, :, 33], 0)
                nc.vector.tensor_scalar_max(
                    Ydpv[:, :, 1:33],
                    m[:].rearrange("p (ho wo) -> p ho wo", ho=16), 0.0)

                # ---- lon conv (along wo, free axis) ----
                m3 = m[:].rearrange("p (ho wo) -> p ho wo", ho=16)
                m23 = m2[:].rearrange("p (ho wo) -> p ho wo", ho=16)
                w3 = lambda t: t.rearrange("p (ho wo) -> p ho wo", ho=16)
                nc.vector.tensor_tensor(m3, w3(vl0), Ydpv[:, :, 0:32], MULT)
                nc.vector.tensor_tensor(m23, w3(vl1), Ydpv[:, :, 1:33], MULT)
                nc.vector.tensor_tensor(m3, m3, m23, ADD)
                nc.vector.tensor_tensor(m23, w3(vl2), Ydpv[:, :, 2:34], MULT)
                nc.vector.tensor_tensor(m3, m3, m23, ADD)
                nc.vector.tensor_tensor(m3, m3, w3(blon), ADD)
                # relu into lat-padded tile Ylp[p, (ho+1)*32 + wo]
                Ylp = work.tile([40, 576], F32)
                nc.gpsimd.memset(Ylp[:, 0:32], 0)
                nc.gpsimd.memset(Ylp[:, 544:576], 0)
                nc.vector.tensor_scalar_max(Ylp[:, 32:544], m[:], 0.0)

                # ---- lat conv (along ho, free axis; contiguous slices) ----
                nc.vector.tensor_tensor(m[:], ul0, Ylp[:, 0:512], MULT)
                nc.vector.tensor_tensor(m2[:], ul1, Ylp[:, 32:544], MULT)
                nc.vector.tensor_tensor(m[:], m[:], m2[:], ADD)
                nc.vector.tensor_tensor(m2[:], ul2, Ylp[:, 64:576], MULT)
                nc.vector.tensor_tensor(m[:], m[:], m2[:], ADD)
                nc.vector.tensor_tensor(m[:], m[:], blat, ADD)

                # ---- upsample h x4 (fused with relu), then w x4 ----
                A = work.tile([40, 2048], F32)   # (ho, hs, wo)
                Av = A[:].rearrange("p (ho hs wo) -> p ho hs wo", ho=16, hs=4)
                mb = m[:].rearrange("p (ho wo) -> p ho wo", ho=16) \
                         .unsqueeze(2).broadcast_to([40, 16, 4, 32])
                nc.scalar.activation(Av, mb, RELU)

                U = outp.tile([40, SLICE], F32)  # (h, wo, ws)
                Uv = U[:].rearrange("p (h wo ws) -> p h wo ws", h=64, ws=4)
                Af = A[:].rearrange("p (h wo) -> p h wo", h=64)
                nc.scalar.copy(Uv[:, :, :, 0], Af)
                nc.scalar.copy(Uv[:, :, :, 1], Af)
                nc.gpsimd.tensor_scalar_add(Uv[:, :, :, 2], Af, 0.0)
                nc.gpsimd.tensor_scalar_add(Uv[:, :, :, 3], Af, 0.0)

                # ---- stores: 3 interleaved depth slices read the same U ----
                for di in range(3):
                    nc.sync.dma_start(
                        bass.AP(y, off + di * SLICE,
                                [[BSTRIDE, 8], [3 * SLICE, 5], [1, SLICE]]),
                        U[:])

    nc.compile()
    return nc


_NC_CACHE = {}


def _get_nc(reps: int = 1):
    if reps not in _NC_CACHE:
        _NC_CACHE[reps] = build_nc(reps)
    return _NC_CACHE[reps]


def kernel(x, w_depth, b_depth, w_lon, b_lon, w_lat, b_lat, reps: int = 1,
           **run_kwargs):
    mm, wts = _pack_consts(w_depth, b_depth, w_lon, b_lon, w_lat, b_lat)
    xf = np.ascontiguousarray(np.asarray(x), dtype=np.float32).reshape(N_CORES, CORE_ELEMS)
    in_maps = [{"x": xf[c], "mm": mm, "wts": wts} for c in range(N_CORES)]
    nc = _get_nc(reps)
    res = run_bass_kernel_spmd(nc, in_maps, core_ids=list(range(N_CORES)), **run_kwargs)
    out = np.stack([r["y"] for r in res.results], axis=0)
    out = out.reshape(B, 15, 64, 128, 1)
    if run_kwargs:
        kernel.last_results = res
    return out


# revision 4
# speedup vs baseline: 1.6271x; 1.6271x over previous
"""Trainium2 Bass kernel for nn_LocalNetwork (avgpool3d -> 3x LocallyConnected1D -> upsample3d).

Sharding: pure data parallelism — batch 256 split as 32 per core across 8 cores.

Per-core layout (B_loc=32 batches, 4 groups of 8):
  partition p = (bl, dslice)  [8 x 15 = 120 partitions]
  Every DMA descriptor covers a 32KB-contiguous DRAM run (one (h,w) slice),
  vs 2KB runs in the row-partitioned layout — this is what the DMA engines
  need to hit full throughput, and it cuts descriptor-generation work on
  the sync engine ~16x.

  - avg-pool over (hs,ws): one fused tensor_reduce XY -> P2 [120, 512]
  - depth pool + depth-conv taps: 3 matmuls [120 -> 40] with 0/(1/48)
    matrices (fuses the /48 mean scale and the +/-1 depth shifts)
  - depth/lon/lat convs: free-axis mult-adds on [40, 512] tiles with
    per-(partition, free) weight tiles precomputed on host
  - upsample: h,w expansion via broadcast copies on scalar/gpsimd;
    depth x3 replication is FREE — three store DMAs read the same
    [40, 8192] SBUF tile into interleaved DRAM depth slices.
"""

import numpy as np

import concourse.bass as bass
import concourse.mybir as mybir
from concourse import bacc
from concourse.bass_utils import run_bass_kernel_spmd
from concourse.tile import TileContext

F32 = mybir.dt.float32
ADD = mybir.AluOpType.add
MULT = mybir.AluOpType.mult
RELU = mybir.ActivationFunctionType.Relu

N_CORES = 8
B = 256
B_CORE = 32          # batches per core
G = 4                # groups per core
B_GRP = 8            # batches per group
CORE_ELEMS = B_CORE * 15 * 64 * 128  # 3,932,160
BSTRIDE = 15 * 64 * 128              # 122,880
SLICE = 64 * 128                     # 8192 elems = one (h,w) plane = 32KB


def _pack_consts(w_depth, b_depth, w_lon, b_lon, w_lat, b_lat):
    """Returns (mm [120,128] f32, wts [40,6144] f32).

    mm: three matmul lhsT tiles [120,40] (cols 0:40 dn / 40:80 mid / 80:120 up)
        out[q=(bl,dp), f] = sum_p lhsT[p=(bl,dsl), q] * P2[p, f]
        coefficient 1/48 folds the avg-pool mean.
    wts: 12 x [40,512] conv weight/bias tiles, p=(bl,dp), f=(ho,wo).
    """
    mm = np.zeros((120, 128), np.float32)
    for bl in range(8):
        for dsl in range(15):
            p = bl * 15 + dsl
            grp = dsl // 3
            for col0, dp in ((0, grp + 1), (40, grp), (80, grp - 1)):
                if 0 <= dp <= 4:
                    mm[p, col0 + bl * 5 + dp] = 1.0 / 48.0

    dp = np.arange(5)[:, None, None]
    ho = np.arange(16)[None, :, None]
    wo = np.arange(32)[None, None, :]
    ld = wo * 112 + ho * 7 + (dp + 1)     # depth seq index (5,16,32)
    ll = dp * 544 + ho * 34 + (wo + 1)    # lon
    lt = dp * 576 + wo * 18 + (ho + 1)    # lat

    def tile(vec, idx):
        t = np.broadcast_to(np.asarray(vec)[idx][None], (8, 5, 16, 32))
        return t.reshape(40, 512)

    cols = []
    for j in range(3):
        cols.append(tile(np.asarray(w_depth)[:, j], ld))
    cols.append(tile(b_depth, ld))
    for j in range(3):
        cols.append(tile(np.asarray(w_lon)[:, j], ll))
    cols.append(tile(b_lon, ll))
    for j in range(3):
        cols.append(tile(np.asarray(w_lat)[:, j], lt))
    cols.append(tile(b_lat, lt))
    wts = np.ascontiguousarray(np.concatenate(cols, axis=1), dtype=np.float32)
    return mm, wts


def build_nc(reps: int = 1) -> bass.Bass:
    nc = bacc.Bacc("TRN2", target_bir_lowering=False, debug=False)
    x = nc.dram_tensor("x", [CORE_ELEMS], F32, kind="ExternalInput")
    mmc = nc.dram_tensor("mm", [120, 128], F32, kind="ExternalInput")
    wtc = nc.dram_tensor("wts", [40, 6144], F32, kind="ExternalInput")
    y = nc.dram_tensor("y", [CORE_ELEMS], F32, kind="ExternalOutput")

    with TileContext(nc) as tc:
        with (
            tc.tile_pool(name="cpool", bufs=1) as cpool,
            tc.tile_pool(name="inp", bufs=2) as inp,
            tc.tile_pool(name="outp", bufs=2) as outp,
            tc.tile_pool(name="work", bufs=2) as work,
            tc.tile_pool(name="psum", bufs=2, space="PSUM") as psum,
        ):
            MM = cpool.tile([120, 128], F32)
            WT = cpool.tile([40, 6144], F32)
            nc.sync.dma_start(MM[:], mmc[:])
            nc.sync.dma_start(WT[:], wtc[:])
            w = lambda i: WT[:, i * 512:(i + 1) * 512]
            wd0, wd1, wd2, bd = (w(i) for i in range(4))
            vl0, vl1, vl2, blon = (w(i) for i in range(4, 8))
            ul0, ul1, ul2, blat = (w(i) for i in range(8, 12))

            for g in range(G * reps):
                g = g % G
                off = g * B_GRP * BSTRIDE

                # ---- load: X[p=(bl,dsl), (h,w)] — 32KB contiguous runs ----
                X = inp.tile([120, SLICE], F32)
                nc.sync.dma_start(
                    X[:], bass.AP(x, off, [[BSTRIDE, 8], [SLICE, 15], [1, SLICE]]))

                # ---- h,w avg-pool (sum): fused reduce over (hs, ws) ----
                P2 = work.tile([120, 512], F32)
                Xv = X[:].rearrange("p (ho hs wo ws) -> p ho wo hs ws",
                                    ho=16, hs=4, wo=32, ws=4)
                nc.vector.tensor_reduce(
                    P2[:].rearrange("p (ho wo) -> p ho wo", ho=16),
                    Xv, mybir.AxisListType.XY, ADD)

                # ---- depth pool (/48) + conv taps via matmul [120->40] ----
                Sdn = psum.tile([40, 512], F32)
                S0 = psum.tile([40, 512], F32)
                Sup = psum.tile([40, 512], F32)
                nc.tensor.matmul(Sdn[:], MM[:, 0:40], P2[:], start=True, stop=True)
                nc.tensor.matmul(S0[:], MM[:, 40:80], P2[:], start=True, stop=True)
                nc.tensor.matmul(Sup[:], MM[:, 80:120], P2[:], start=True, stop=True)

                # ---- depth conv ----
                m = work.tile([40, 512], F32)
                m2 = work.tile([40, 512], F32)
                nc.vector.tensor_tensor(m[:], wd0, Sdn[:], MULT)
                nc.vector.tensor_tensor(m2[:], wd1, S0[:], MULT)
                nc.vector.tensor_tensor(m[:], m[:], m2[:], ADD)
                nc.vector.tensor_tensor(m2[:], wd2, Sup[:], MULT)
                nc.vector.tensor_tensor(m[:], m[:], m2[:], ADD)
                nc.vector.tensor_tensor(m[:], m[:], bd, ADD)
                # relu into lon-padded tile Ydp[p, ho*34 + (wo+1)]
                Ydp = work.tile([40, 544], F32)
                Ydpv = Ydp[:].rearrange("p (ho wp) -> p ho wp", ho=16, wp=34)
                nc.gpsimd.memset(Ydpv[:, :, 0], 0)
                nc.gpsimd.memset(Ydpv[:, :, 33], 0)
                nc.vector.tensor_scalar_max(
                    Ydpv[:, :, 1:33],
                    m[:].rearrange("p (ho wo) -> p ho wo", ho=16), 0.0)

                # ---- lon conv (along wo, free axis) ----
                m3 = m[:].rearrange("p (ho wo) -> p ho wo", ho=16)
                m23 = m2[:].rearrange("p (ho wo) -> p ho wo", ho=16)
                w3 = lambda t: t.rearrange("p (ho wo) -> p ho wo", ho=16)
                nc.vector.tensor_tensor(m3, w3(vl0), Ydpv[:, :, 0:32], MULT)
                nc.vector.tensor_tensor(m23, w3(vl1), Ydpv[:, :, 1:33], MULT)
                nc.vector.tensor_tensor(m3, m3, m23, ADD)
                nc.vector.tensor_tensor(m23, w3(vl2), Ydpv[:, :, 2:34], MULT)
                nc.vector.tensor_tensor(m3, m3, m23, ADD)
                nc.vector.tensor_tensor(m3, m3, w3(blon), ADD)
                # relu into lat-padded tile Ylp[p, (ho+1)*32 + wo]
                Ylp = work.tile([40, 576], F32)
                nc.gpsimd.memset(Ylp[:, 0:32], 0)
                nc.gpsimd.memset(Ylp[:, 544:576], 0)
                nc.vector.tensor_scalar_max(Ylp[:, 32:544], m[:], 0.0)

                # ---- lat conv (along ho, free axis; contiguous slices) ----
                nc.vector.tensor_tensor(m[:], ul0, Ylp[:, 0:512], MULT)
                nc.vector.tensor_tensor(m2[:], ul1, Ylp[:, 32:544], MULT)
                nc.vector.tensor_tensor(m[:], m[:], m2[:], ADD)
                nc.vector.tensor_tensor(m2[:], ul2, Ylp[:, 64:576], MULT)
                nc.vector.tensor_tensor(m[:], m[:], m2[:], ADD)
                nc.vector.tensor_tensor(m[:], m[:], blat, ADD)

                # ---- upsample h x4 (fused with relu), then w x4 ----
                A = work.tile([40, 2048], F32)   # (ho, hs, wo)
                Av = A[:].rearrange("p (ho hs wo) -> p ho hs wo", ho=16, hs=4)
                mb = m[:].rearrange("p (ho wo) -> p ho wo", ho=16) \
                         .unsqueeze(2).broadcast_to([40, 16, 4, 32])
                nc.scalar.activation(Av, mb, RELU)

                U = outp.tile([40, SLICE], F32)  # (h, wo, ws)
                Uv = U[:].rearrange("p (h wo ws) -> p h wo ws", h=64, ws=4)
                Ab = A[:].rearrange("p (h wo) -> p h wo", h=64) \
                         .unsqueeze(3).broadcast_to([40, 64, 32, 4])
                nc.vector.tensor_scalar_add(Uv, Ab, 0.0)

                # ---- stores: 3 interleaved depth slices read the same U ----
                for di in range(3):
                    nc.sync.dma_start(
                        bass.AP(y, off + di * SLICE,
                                [[BSTRIDE, 8], [3 * SLICE, 5], [1, SLICE]]),
                        U[:])

    nc.compile()
    return nc


_NC_CACHE = {}


def _get_nc(reps: int = 1):
    if reps not in _NC_CACHE:
        _NC_CACHE[reps] = build_nc(reps)
    return _NC_CACHE[reps]


def kernel(x, w_depth, b_depth, w_lon, b_lon, w_lat, b_lat, reps: int = 1,
           **run_kwargs):
    mm, wts = _pack_consts(w_depth, b_depth, w_lon, b_lon, w_lat, b_lat)
    xf = np.ascontiguousarray(np.asarray(x), dtype=np.float32).reshape(N_CORES, CORE_ELEMS)
    in_maps = [{"x": xf[c], "mm": mm, "wts": wts} for c in range(N_CORES)]
    nc = _get_nc(reps)
    res = run_bass_kernel_spmd(nc, in_maps, core_ids=list(range(N_CORES)), **run_kwargs)
    out = np.stack([r["y"] for r in res.results], axis=0)
    out = out.reshape(B, 15, 64, 128, 1)
    if run_kwargs:
        kernel.last_results = res
    return out
